# revision 6
# baseline (speedup 1.0000x reference)
"""Bass/Tile Trainium2 kernel for nn_Attention_VK (dense transformer
attention with learned prompt KV tokens), data-parallel over batch across 8
NeuronCores. Measured ~0.62-0.64 ms/iteration per core (baseline: 1.02 ms).

Shapes (hardcoded): x[32,785,768], qkv_w[2304,768], proj_w[768,768],
proj_b[768], prompt_kv[12,50,64]. Output [32,785,768] fp32. Per core: 4
batches. Host pre-packs transposed bf16 layouts (x^T, W^T per 128-row
contraction chunk) so the device does no transposes; all matmul operands are
bf16 (1 PE col/cycle at any width; the 2e-2 tolerance absorbs quantization,
measured rel err 4.1e-3), PSUM accumulation fp32.

Key HW findings baked into the schedule (from perfetto/microbench evidence):
- Matmuls with contraction < 128 run ~2x slower than their column count on
  TRN2 (449 vs 213 ns for c=64, 512 cols). QK therefore contracts over all
  128 partitions using zero-padded K tiles (head hh's K in rows 64*hh..+64,
  other rows permanently zero) against the full 128-row q-pair.
- Tight cross-engine chains cost ~1-3 us each on HW (semaphore+queue
  latency), so every consumer gets structural slack: scores^T = K q^T in
  [keys, queries] layout; exp (ScalarE, scale=1/8 folded, max-subtract
  skipped -- scores are O(1)) writes a 14-deep bf16 A-tile ring; the AV
  matmuls of head h-1 run interleaved 2-per-step under head h's QK stream;
  out-projection of batch b-1 and v-projection of batch b+1 are chopped into
  PSUM-bank-sized groups and injected as filler so the in-order PE queue
  never blocks on a waiting instruction.
- Softmax denominator rides the AV matmul as a ones column in V (PSUM row
  64); normalization per query-half [65,393] (single PSUM bank): evacuate
  o+denominator to SBUF (one PSUM read), then DVE reciprocal -> GpSimd
  partition_broadcast -> DVE multiply writes attnT, all off the PSUM port.
- qk-projection streams bank 0 (cols 0:512) for q AND k before bank 1, so
  the PSUM->SBUF copies of bank 0 overlap bank 1's matmuls.
- PSUM budget exactly 8 banks: scores 2x2 + AV-halves 2x1 + filler 2x1.
"""

import numpy as np

B, N, C = 32, 785, 768
H, D, P = 12, 64, 50
M = N + P
NCORES = 8
NB = B // NCORES
CC = C // 128
KT = (M + 127) // 128
TT = (N + 127) // 128
VW = H * (D + 1)
N2 = N + 1          # 786
NH = N2 // 2        # 393 query half

INPUT_NAMES = ["xt", "wqkvt", "pwt", "biasb", "pk", "pv"]


def _build(nc, loop_n=1, variant='full'):
    import contextlib

    import concourse.mybir as mybir
    import concourse.tile as tile

    f32 = mybir.dt.float32
    bf16 = mybir.dt.bfloat16

    xt = nc.dram_tensor("xt", [NB, 128, CC, N], bf16, kind="ExternalInput").ap()
    wqkvt = nc.dram_tensor("wqkvt", [128, CC, 3 * C], bf16, kind="ExternalInput").ap()
    pwt = nc.dram_tensor("pwt", [128, CC, C], bf16, kind="ExternalInput").ap()
    biasb = nc.dram_tensor("biasb", [128, C], f32, kind="ExternalInput").ap()
    pk = nc.dram_tensor("pk", [128, CC, P], bf16, kind="ExternalInput").ap()
    pv = nc.dram_tensor("pv", [P, VW], bf16, kind="ExternalInput").ap()
    out = nc.dram_tensor("out", [NB * N, C], f32, kind="ExternalOutput").ap()

    with tile.TileContext(nc) as tc:
        with (
            tc.tile_pool(name="const", bufs=1) as const,
            tc.tile_pool(name="xp", bufs=2) as xp,
            tc.tile_pool(name="atp", bufs=2) as atp,
            tc.tile_pool(name="qkp", bufs=2) as qkp,
            tc.tile_pool(name="vp", bufs=2) as vp,
            tc.tile_pool(name="ap", bufs=16) as apool,
            tc.tile_pool(name="small", bufs=2) as small,
            tc.tile_pool(name="psA", bufs=2, space="PSUM") as psA,
            tc.tile_pool(name="psO", bufs=2, space="PSUM") as psO,
            tc.tile_pool(name="psF", bufs=2, space="PSUM") as psF,
        ):
            w_sb = const.tile([128, CC, 3 * C], bf16)
            nc.scalar.dma_start(out=w_sb[:, :, 2 * C:3 * C],
                                in_=wqkvt[:, :, 2 * C:3 * C])
            nc.sync.dma_start(out=w_sb[:, :, 0:C], in_=wqkvt[:, :, 0:C])
            nc.sync.dma_start(out=w_sb[:, :, C:2 * C], in_=wqkvt[:, :, C:2 * C])
            pw_sb = const.tile([128, CC, C], bf16)
            nc.scalar.dma_start(out=pw_sb, in_=pwt)
            bias_sb = const.tile([128, C], f32)
            nc.scalar.dma_start(out=bias_sb, in_=biasb)
            pk_sb = const.tile([128, CC, P], bf16)
            nc.scalar.dma_start(out=pk_sb, in_=pk)
            # Zero-padded K tiles: head hh's K in rows 64*hh..64*hh+64, the
            # other 64 rows stay zero forever. QK then contracts over all 128
            # partitions (c=128 matmuls run ~2x faster than c=64 on HW), the
            # zero half contributing nothing.
            k0_sb = const.tile([128, M], bf16)
            k1_sb = const.tile([128, M], bf16)
            nc.vector.memset(k0_sb[64:128, :], 0.0)
            nc.vector.memset(k1_sb[0:64, :], 0.0)
            a_const = None
            if 'constA' in VARIANT_FLAGS[variant]:
                a_const = const.tile([128, N2], bf16)
                nc.vector.memset(a_const, 0.001)

            loop = (tc.For_i(0, loop_n, 1) if loop_n > 1
                    else contextlib.nullcontext())
            with loop:
                _emit_body(nc, tc, f32, bf16, mybir, xt, pk_sb, pv, out,
                           w_sb, pw_sb, bias_sb, xp, atp, qkp, vp, apool,
                           small, psA, psO, psF, variant, k0_sb, k1_sb,
                           a_const)
    return nc


VARIANT_FLAGS = {
    'full': ('oscopy', 'qksplit'),
    'fill6': ('oscopy', 'fill6'),
    'alt': ('alt',),
    'nochain': ('nochain',),
    'noav': ('nochain', 'noav'),
    'noexp': ('nochain', 'noav', 'noexp'),
    'noqk': ('nochain', 'noav', 'noexp', 'noqk'),
    'noproj': ('noproj',),
    'nofill': ('nofill',),   # fillers emitted as solid blocks, not interleaved
    'mmonly': ('nochain', 'noexp', 'constA'),  # full matmul stream, no exp
}


def _emit_body(nc, tc, f32, bf16, mybir, xt, pk_sb, pv, out, w_sb, pw_sb,
               bias_sb, xp, atp, qkp, vp, apool, small, psA, psO, psF,
               variant='full', k0_sb=None, k1_sb=None, a_const=None):
    Exp = mybir.ActivationFunctionType.Exp
    FL = VARIANT_FLAGS[variant]
    k_pair = (k0_sb, k1_sb)

    def mm(out_ap, lhsT, rhs, start, stop):
        nc.tensor.matmul(out_ap, lhsT=lhsT, rhs=rhs, start=start, stop=stop)

    xT_tiles = {}
    v_tiles = {}
    at_tiles = {}

    def fetch_x(b):
        xT = xp.tile([128, CC, N2], bf16, tag="x", name=f"xT{b}")
        nc.gpsimd.dma_start(out=xT[:, :, 0:N], in_=xt[b])
        nc.gpsimd.memset(xT[:, :, N:N2], 0.0)
        xT_tiles[b] = xT

    def new_v(b):
        v_sb = vp.tile([128, KT, VW], bf16, tag="v", name=f"v{b}")
        nc.sync.dma_start(out=v_sb[17:17 + P, KT - 1, :], in_=pv)
        v_tiles[b] = v_sb
        return v_sb

    def vproj_units(b):
        """Yield emit-callables: per (tt,g): 6 accumulating mms + evict."""
        v_sb = new_v(b)
        xT = xT_tiles[b]
        for tt in range(TT):
            tl = min(128, N - tt * 128)
            for g in range(2):
                ps = psF.tile([128, 384], f32, tag="f")
                for cc in range(CC):
                    yield (lambda ps=ps, tl=tl, tt=tt, g=g, cc=cc, xT=xT: mm(
                        ps[:tl, :],
                        xT[:, cc, tt * 128:tt * 128 + tl],
                        w_sb[:, cc, 2 * C + g * 384:2 * C + (g + 1) * 384],
                        (cc == 0), (cc == CC - 1)))

                def evict(ps=ps, tl=tl, tt=tt, g=g, v_sb=v_sb):
                    vh = v_sb[:tl, tt, :].rearrange("p (h e) -> p h e", e=D + 1)
                    nc.vector.tensor_copy(
                        vh[:, 6 * g:6 * g + 6, 0:D],
                        ps[:tl, :].rearrange("p (h d) -> p h d", d=D))
                    nc.vector.memset(vh[:, 6 * g:6 * g + 6, D:D + 1], 1.0)
                yield evict

    def oproj_units(b):
        """Output projection of batch b from attnT(b), in 384-col groups."""
        attnT = at_tiles.pop(b)
        for tt in range(TT):
            tl = min(128, N - tt * 128)
            for g in range(2):
                ps = psF.tile([128, 384], f32, tag="f")
                for cc in range(CC):
                    yield (lambda ps=ps, tl=tl, tt=tt, g=g, cc=cc,
                           attnT=attnT: mm(
                        ps[:tl, :],
                        attnT[:, cc, tt * 128:tt * 128 + tl],
                        pw_sb[:, cc, g * 384:(g + 1) * 384],
                        (cc == 0), (cc == CC - 1)))

                def evict(ps=ps, tl=tl, tt=tt, g=g, b=b):
                    o_sb = small.tile([128, 384], f32, tag="osb")
                    nc.vector.tensor_add(o_sb[:tl], ps[:tl],
                                         bias_sb[:tl, g * 384:(g + 1) * 384])
                    nc.sync.dma_start(
                        out=out[b * N + tt * 128: b * N + tt * 128 + tl,
                                g * 384:(g + 1) * 384],
                        in_=o_sb[:tl])
                yield evict

    def interleave(*gens):
        gens = [g for g in gens if g is not None]
        while gens:
            nxt = []
            for g in gens:
                try:
                    yield next(g)
                    nxt.append(g)
                except StopIteration:
                    pass
            gens = nxt

    def av_units(h, hp, r0, b):
        """AV of head h (reads a_tiles[(h, kt)]) + normalization chains."""
        v_sb = v_tiles[b]
        attnT = at_tiles[b]
        for half in range(2):
            o_ps = psO.tile([D + 1, NH], f32, tag="o", name=f"o{b}_{h}_{half}")
            for kt in range(KT):
                yield (lambda o_ps=o_ps, kt=kt, half=half, h=h: mm(
                    o_ps,
                    v_tiles[b][:min(128, M - kt * 128), kt,
                               h * (D + 1):(h + 1) * (D + 1)],
                    a_tiles[(h, kt)][:min(128, M - kt * 128),
                                     half * NH:(half + 1) * NH],
                    (kt == 0), (kt == KT - 1)))

            def chain(o_ps=o_ps, half=half, hp=hp, r0=r0, attnT=attnT):
                hw = NH if half == 0 else N - NH
                src_o = o_ps
                if 'oscopy' in FL:
                    os = small.tile([D + 1, NH], f32, tag="os")
                    nc.vector.tensor_copy(os, o_ps)
                    src_o = os
                rec = small.tile([1, NH], f32, tag="rec")
                nc.vector.reciprocal(rec, src_o[D:D + 1, :])
                recb = small.tile([D, NH], f32, tag="recb")
                nc.gpsimd.partition_broadcast(recb, rec, channels=D)
                nc.vector.tensor_mul(
                    attnT[r0:r0 + D, hp, half * NH:half * NH + hw],
                    src_o[0:D, 0:hw], recb[:, 0:hw])
            if 'nochain' not in FL:
                yield chain

    a_tiles = {}

    def emit_some(it, n):
        cnt = 0
        if it is None:
            return
        while cnt < n:
            try:
                next(it)()
            except StopIteration:
                return
            cnt += 1

    def drain(it):
        if it is None:
            return
        for unit in it:
            unit()

    # ---------- schedule ----------
    fetch_x(0)
    fetch_x(1)
    drain(vproj_units(0))            # prologue: v(0) solid
    av_pend = None

    for b in range(NB):
        if b + 2 < NB:
            fetch_x(b + 2)
        # AV of the previous batch's last head must fully drain (incl. its
        # normalization chains) before oproj fillers of b-1 enter the
        # in-order PE queue, else the queue wedges on attnT(b-1).
        drain(av_pend)
        av_pend = None

        attnT = atp.tile([128, CC, N], bf16, tag="at", name=f"attnT{b}")
        at_tiles[b] = attnT
        if 'nochain' in FL:
            nc.vector.memset(attnT, 0.0)

        fillers = []
        if b > 0:
            if 'noproj' not in FL:
                fillers.append(oproj_units(b - 1))
            else:
                at_tiles.pop(b - 1)
        if b + 1 < NB:
            fillers.append(vproj_units(b + 1))
        filler = interleave(*fillers) if fillers else None
        if 'nofill' in FL:
            drain(filler)
            filler = None
        xT = xT_tiles[b]

        for hp in range(CC):
            q_sb = qkp.tile([128, N2], bf16, tag="q", name=f"q{b}_{hp}")
            nc.vector.tensor_copy(k0_sb[0:64, N:M], pk_sb[0:64, hp, :])
            nc.vector.tensor_copy(k1_sb[64:128, N:M], pk_sb[64:128, hp, :])
            if 'qksplit' in FL:
                # bank-split order: all part-1 (cols 0:512) mms for q AND k
                # first, then their copies run while part-2 mms stream, so
                # the first QK only waits on the short part-2 copies
                ps_q = psA.tile([128, N2], f32, tag="mm")
                ps_k = psA.tile([128, N2], f32, tag="mm")
                for n0 in (0, 512):
                    nl = min(512, N2 - n0)
                    for ps, fbase in ((ps_q, hp * 128), (ps_k, C + hp * 128)):
                        for cc in range(CC):
                            mm(ps[:, n0:n0 + nl],
                               w_sb[:, cc, fbase:fbase + 128],
                               xT[:, cc, n0:n0 + nl],
                               (cc == 0), (cc == CC - 1))
                    ne = min(n0 + nl, N)
                    nc.vector.tensor_copy(q_sb[:, n0:n0 + nl],
                                          ps_q[:, n0:n0 + nl])
                    nc.vector.tensor_copy(k0_sb[0:64, n0:ne],
                                          ps_k[0:64, n0:ne])
                    nc.vector.tensor_copy(k1_sb[64:128, n0:ne],
                                          ps_k[64:128, n0:ne])
            else:
                k_ps = None
                for dst_q, fbase in ((True, hp * 128), (False, C + hp * 128)):
                    ps = psA.tile([128, N2], f32, tag="mm")
                    for cc in range(CC):
                        for n0 in range(0, N2, 512):
                            nl = min(512, N2 - n0)
                            mm(ps[:, n0:n0 + nl],
                               w_sb[:, cc, fbase:fbase + 128],
                               xT[:, cc, n0:n0 + nl],
                               (cc == 0), (cc == CC - 1))
                    if dst_q:                  # q done: copy while k runs
                        nc.vector.tensor_copy(q_sb, ps)
                    else:
                        k_ps = ps
                nc.vector.tensor_copy(k0_sb[0:64, 0:N], k_ps[0:64, 0:N])
                nc.vector.tensor_copy(k1_sb[64:128, 0:N], k_ps[64:128, 0:N])
            emit_some(filler, 6 if 'fill6' in FL else 4)

            for hh in range(2):
                h = 2 * hp + hh
                r0 = 64 * hh
                for kt in range(KT):
                    kl = min(128, M - kt * 128)
                    if 'noqk' not in FL:
                        s_ps = psA.tile([128, N2], f32, tag="mm")
                        for n0 in range(0, N2, 512):
                            nl = min(512, N2 - n0)
                            mm(s_ps[:kl, n0:n0 + nl],
                               k_pair[hh][:, kt * 128:kt * 128 + kl],
                               q_sb[:, n0:n0 + nl],
                               True, True)
                    emit_some(av_pend, 3 if 'av3' in FL else 2)
                    if 'constA' in FL:
                        a_tiles[(h, kt)] = a_const
                    if 'noexp' not in FL:
                        a_sb = apool.tile([128, N2], bf16, tag="A")
                        a_tiles[(h, kt)] = a_sb
                        if 'alt' in FL and kt % 2 == 0:
                            # route scores via SBUF on even steps: halves the
                            # ACT PSUM reads contending with PE PSUM writes
                            s_sb = small.tile([128, N2], bf16, tag="S")
                            nc.vector.tensor_copy(s_sb[:kl, :], s_ps[:kl, :])
                            nc.scalar.activation(
                                a_sb[:kl, :], s_sb[:kl, :], Exp,
                                scale=D ** -0.5)
                        else:
                            nc.scalar.activation(
                                a_sb[:kl, :], s_ps[:kl, :], Exp,
                                scale=D ** -0.5)
                    emit_some(filler, 2)
                drain(av_pend)
                av_pend = (av_units(h, hp, r0, b)
                           if 'noav' not in FL else None)
        drain(filler)

    drain(av_pend)                     # AV of head 11, batch 3
    if 'noproj' not in FL:
        drain(oproj_units(NB - 1))     # epilogue: out-proj(3) solid
    else:
        at_tiles.pop(NB - 1)


def _pack_inputs(x, qkv_w, proj_w, proj_b, prompt_kv):
    import ml_dtypes
    bf = ml_dtypes.bfloat16
    x = np.ascontiguousarray(np.asarray(x, dtype=np.float32))
    qkv_w = np.asarray(qkv_w, dtype=np.float32)
    proj_w = np.asarray(proj_w, dtype=np.float32)
    proj_b = np.asarray(proj_b, dtype=np.float32)
    prompt_kv = np.asarray(prompt_kv, dtype=np.float32)

    xt = np.ascontiguousarray(
        x.reshape(NCORES, NB, N, CC, 128).transpose(0, 1, 4, 3, 2)).astype(bf)
    wqkvt = np.ascontiguousarray(
        qkv_w.T.reshape(CC, 128, 3 * C).transpose(1, 0, 2)).astype(bf)
    pwt = np.ascontiguousarray(
        proj_w.T.reshape(CC, 128, C).transpose(1, 0, 2)).astype(bf)
    biasb = np.ascontiguousarray(np.broadcast_to(proj_b, (128, C)))
    pk = np.ascontiguousarray(
        prompt_kv.transpose(0, 2, 1).reshape(CC, 128, P).transpose(1, 0, 2)
    ).astype(bf)
    pv = np.zeros((P, VW), dtype=np.float32)
    for h in range(H):
        pv[:, h * (D + 1):h * (D + 1) + D] = prompt_kv[h]
        pv[:, h * (D + 1) + D] = 1.0
    pv = pv.astype(bf)
    return xt, wqkvt, pwt, biasb, pk, pv


def run(x, qkv_w, proj_w, proj_b, prompt_kv, trace=False):
    from concourse import bacc
    from concourse.bass_utils import run_bass_kernel_spmd

    packed = _pack_inputs(x, qkv_w, proj_w, proj_b, prompt_kv)
    nc = bacc.Bacc("TRN2", debug=False, num_devices=NCORES)
    _build(nc)
    nc.compile()

    shared = dict(zip(INPUT_NAMES[1:], packed[1:]))
    in_maps = [dict(shared, xt=packed[0][i]) for i in range(NCORES)]
    res = run_bass_kernel_spmd(
        nc, in_maps, core_ids=list(range(NCORES)), trace=trace)
    outs = [res.results[i]["out"].reshape(NB, N, C) for i in range(NCORES)]
    full = np.concatenate(outs, axis=0)
    return full, res


def kernel(x, qkv_w, proj_w, proj_b, prompt_kv):
    full, _ = run(x, qkv_w, proj_w, proj_b, prompt_kv)
    return full
